# revision 1
# baseline (speedup 1.0000x reference)
"""Windowed attention (swin-style, 49-token windows, 8 heads) with DynamicPosBias.

Strategy: data-parallel over B=2048 windows -> 256 windows/core on 8 cores.
Host pre-transposes q,k per head to [W, 8, 64, 49] so the device needs no
transposes; v gets a fused ones-column so PV matmul also produces the softmax
denominator. Position-bias MLP runs on device once; the bias is fused into the
QK matmul as 49 extra contraction rows (lhsT=[K^T;I49], rhs=[Q^T;8*rpb]) and
exp(0.125*S) folds the 1/sqrt(64) scale.
"""

import numpy as np
from contextlib import ExitStack

import concourse.bass as bass
import concourse.mybir as mybir
import concourse.tile as tile
from concourse import bacc
from concourse.bass_utils import run_bass_kernel_spmd

G = 7
NTOK = 49          # tokens per window
H = 8              # heads
HD = 64            # head dim
C = 512
B = 2048
NCORES = 8
W = B // NCORES    # windows per core
T = (2 * G - 1) ** 2  # 169 bias table entries
PDIM = 32          # MLP hidden
NBUF = 4
F32 = mybir.dt.float32
F16 = mybir.dt.float16
I32 = mybir.dt.int32


def _rel_idx():
    coords = np.stack(np.meshgrid(np.arange(G), np.arange(G), indexing="ij")).reshape(2, -1)
    rel = (coords[:, :, None] - coords[:, None, :]).transpose(1, 2, 0)
    rel = rel.copy()
    rel[:, :, 0] += G - 1
    rel[:, :, 1] += G - 1
    rel[:, :, 0] *= 2 * G - 1
    return rel.sum(-1)  # [i, j] in [0, 169)


def _biases_t():
    pb = np.arange(1 - G, G, dtype=np.float32)
    b = np.stack(np.meshgrid(pb, pb, indexing="ij")).reshape(2, -1)  # [2, 169]
    return np.ascontiguousarray(b.astype(np.float32))


_CACHED_NC = None
LAST_RESULTS = None


def _build_nc():
    global _CACHED_NC
    if _CACHED_NC is not None:
        return _CACHED_NC
    nc = bacc.Bacc(None, target_bir_lowering=False)

    qt_d = nc.dram_tensor("qt", [W, H, HD, NTOK], F16, kind="ExternalInput")
    kt_d = nc.dram_tensor("kt", [W, H, HD, NTOK], F16, kind="ExternalInput")
    v_d = nc.dram_tensor("vaug", [W, NTOK, H * 65], F16, kind="ExternalInput")
    id8_d = nc.dram_tensor("ident8", [NTOK, H * NTOK], F16, kind="ExternalInput")
    ridx_d = nc.dram_tensor("relidx", [NTOK, NTOK], I32, kind="ExternalInput")
    bia_d = nc.dram_tensor("biases_t", [2, T], F32, kind="ExternalInput")
    ppw_d = nc.dram_tensor("pos_proj_w", [2, PDIM], F32, kind="ExternalInput")
    ppb_d = nc.dram_tensor("pos_proj_b", [PDIM], F32, kind="ExternalInput")
    mlp_vec = {}
    for nm in ["ln1_g", "ln1_b", "b1", "ln2_g", "ln2_b", "b2", "ln3_g", "ln3_b"]:
        mlp_vec[nm] = nc.dram_tensor(nm, [PDIM], F32, kind="ExternalInput")
    w1_d = nc.dram_tensor("w1", [PDIM, PDIM], F32, kind="ExternalInput")
    w2_d = nc.dram_tensor("w2", [PDIM, PDIM], F32, kind="ExternalInput")
    w3_d = nc.dram_tensor("w3", [PDIM, H], F32, kind="ExternalInput")
    b3_d = nc.dram_tensor("b3", [H], F32, kind="ExternalInput")
    out_d = nc.dram_tensor("out", [W, NTOK, C], F32, kind="ExternalOutput")
    pos_dram = nc.dram_tensor("pos_scratch", [T, H], F32, kind="Internal")

    with tile.TileContext(nc) as tc, ExitStack() as ctx:
        const = ctx.enter_context(tc.tile_pool(name="const", bufs=1))
        mlp = ctx.enter_context(tc.tile_pool(name="mlp", bufs=1))
        mps = ctx.enter_context(tc.tile_pool(name="mps", bufs=1, space="PSUM"))

        # ---------- DynamicPosBias MLP: X^T layout [feat, 169] ----------
        biasesT = mlp.tile([2, T], F32, tag="biasesT")
        nc.sync.dma_start(biasesT[:], bia_d[:])
        ppw = mlp.tile([2, PDIM], F32, tag="ppw")
        nc.sync.dma_start(ppw[:], ppw_d[:])
        vec_sb = {}
        for nm in ["ln1_g", "ln1_b", "b1", "ln2_g", "ln2_b", "b2", "ln3_g", "ln3_b"]:
            t = mlp.tile([PDIM, 1], F32, tag=nm)
            nc.sync.dma_start(t[:], mlp_vec[nm][:])
            vec_sb[nm] = t
        ppb = mlp.tile([PDIM, 1], F32, tag="ppb")
        nc.sync.dma_start(ppb[:], ppb_d[:])
        w1 = mlp.tile([PDIM, PDIM], F32, tag="w1")
        nc.sync.dma_start(w1[:], w1_d[:])
        w2 = mlp.tile([PDIM, PDIM], F32, tag="w2")
        nc.sync.dma_start(w2[:], w2_d[:])
        w3 = mlp.tile([PDIM, H], F32, tag="w3")
        nc.sync.dma_start(w3[:], w3_d[:])
        b3 = mlp.tile([H, 1], F32, tag="b3")
        nc.sync.dma_start(b3[:], b3_d[:])

        eps_t = mlp.tile([1, 1], F32, tag="eps")
        nc.gpsimd.memset(eps_t[:], 1e-5)
        ones_inv = mlp.tile([PDIM, 1], F32, tag="ones_inv")
        nc.gpsimd.memset(ones_inv[:], 1.0 / PDIM)
        ones_row = mlp.tile([1, PDIM], F32, tag="ones_row")
        nc.gpsimd.memset(ones_row[:], 1.0)

        x_ps = mps.tile([PDIM, T], F32, tag="mpsA")
        nc.tensor.matmul(out=x_ps[:], lhsT=ppw[:], rhs=biasesT[:], start=True, stop=True)
        x_sb = mlp.tile([PDIM, T], F32, tag="x_sb")
        nc.vector.tensor_scalar_add(out=x_sb[:], in0=x_ps[:], scalar1=ppb[:])

        layer_params = [
            (vec_sb["ln1_g"], vec_sb["ln1_b"], w1, vec_sb["b1"], PDIM),
            (vec_sb["ln2_g"], vec_sb["ln2_b"], w2, vec_sb["b2"], PDIM),
            (vec_sb["ln3_g"], vec_sb["ln3_b"], w3, b3, H),
        ]
        for li, (g_ap, bln_ap, w_ap, bout_ap, odim) in enumerate(layer_params):
            mu_ps = mps.tile([1, T], F32, tag="mpsA")
            nc.tensor.matmul(out=mu_ps[:], lhsT=ones_inv[:], rhs=x_sb[:], start=True, stop=True)
            mu_sb = mlp.tile([1, T], F32, tag=f"mus{li}")
            nc.vector.tensor_copy(mu_sb[:], mu_ps[:])
            mub_ps = mps.tile([PDIM, T], F32, tag="mpsA")
            nc.tensor.matmul(out=mub_ps[:], lhsT=ones_row[:], rhs=mu_sb[:], start=True, stop=True)
            xc = mlp.tile([PDIM, T], F32, tag=f"xc{li}")
            nc.vector.tensor_tensor(out=xc[:], in0=x_sb[:], in1=mub_ps[:], op=mybir.AluOpType.subtract)
            sq = mlp.tile([PDIM, T], F32, tag=f"sq{li}")
            nc.vector.tensor_tensor(out=sq[:], in0=xc[:], in1=xc[:], op=mybir.AluOpType.mult)
            var_ps = mps.tile([1, T], F32, tag="mpsA")
            nc.tensor.matmul(out=var_ps[:], lhsT=ones_inv[:], rhs=sq[:], start=True, stop=True)
            sd = mlp.tile([1, T], F32, tag=f"sd{li}")
            nc.scalar.activation(sd[:], var_ps[:], mybir.ActivationFunctionType.Sqrt, bias=eps_t[:])
            rstd = mlp.tile([1, T], F32, tag=f"rstd{li}")
            nc.vector.reciprocal(rstd[:], sd[:])
            rstdb_ps = mps.tile([PDIM, T], F32, tag="mpsA")
            nc.tensor.matmul(out=rstdb_ps[:], lhsT=ones_row[:], rhs=rstd[:], start=True, stop=True)
            xh = mlp.tile([PDIM, T], F32, tag=f"xh{li}")
            nc.vector.tensor_tensor(out=xh[:], in0=xc[:], in1=rstdb_ps[:], op=mybir.AluOpType.mult)
            hrelu = mlp.tile([PDIM, T], F32, tag=f"hr{li}")
            nc.scalar.activation(hrelu[:], xh[:], mybir.ActivationFunctionType.Relu,
                                 bias=bln_ap[:], scale=g_ap[:])
            xn_ps = mps.tile([odim, T], F32, tag="mpsA")
            nc.tensor.matmul(out=xn_ps[:], lhsT=w_ap[:], rhs=hrelu[:], start=True, stop=True)
            x_sb = mlp.tile([odim, T], F32, tag=f"xsb{li}")
            nc.vector.tensor_scalar_add(out=x_sb[:], in0=xn_ps[:], scalar1=bout_ap[:])

        # x_sb is now pos^T [8, 169]; push to DRAM as [169, 8] (slow tiny DMA)
        nc.sync.dma_start(pos_dram[:].rearrange("t (h o) -> h t o", o=1), x_sb[:])

        # ---------- gather rpb: 49 row-gathers -> [49, (i,h)] then reorder ----------
        ridx_sb = const.tile([NTOK, NTOK], I32, tag="ridx")
        nc.sync.dma_start(ridx_sb[:], ridx_d[:])
        rpb_tmp = const.tile([NTOK, NTOK * H], F32, tag="rpb_tmp")
        for i in range(NTOK):
            nc.gpsimd.indirect_dma_start(
                out=rpb_tmp[:, H * i : H * i + H],
                out_offset=None,
                in_=pos_dram[:],
                in_offset=bass.IndirectOffsetOnAxis(ap=ridx_sb[:, i : i + 1], axis=0),
            )
        rpb_sb = const.tile([NTOK, H * NTOK], F16, tag="rpb_sb")
        nc.vector.tensor_scalar_mul(
            out=rpb_sb[:].rearrange("p (h i) -> p h i", h=H),
            in0=rpb_tmp[:].rearrange("p (i h) -> p h i", h=H),
            scalar1=8.0,
        )

        # ---------- persistent per-slot QT/KT buffers ----------
        qt_slots = [const.tile([128, H * NTOK], F16, tag=f"qts{s}", name=f"qts{s}") for s in range(NBUF)]
        kt_slots = [const.tile([128, H * NTOK], F16, tag=f"kts{s}", name=f"kts{s}") for s in range(NBUF)]
        for s in range(NBUF):
            nc.sync.dma_start(qt_slots[s][HD : HD + NTOK, :], rpb_sb[:, :])
            nc.sync.dma_start(kt_slots[s][HD : HD + NTOK, :], id8_d[:])

        vpool = ctx.enter_context(tc.tile_pool(name="vpool", bufs=NBUF))
        epool = ctx.enter_context(tc.tile_pool(name="epool", bufs=3))
        opool = ctx.enter_context(tc.tile_pool(name="opool", bufs=3))
        rpool = ctx.enter_context(tc.tile_pool(name="rpool", bufs=3))
        stps = ctx.enter_context(tc.tile_pool(name="stps", bufs=2, space="PSUM"))
        pvps = ctx.enter_context(tc.tile_pool(name="pvps", bufs=2, space="PSUM"))

        for w in range(W):
            s = w % NBUF
            nc.sync.dma_start(
                qt_slots[s][0:HD, :].rearrange("d (h i) -> d h i", h=H),
                qt_d[w].rearrange("h d i -> d h i"),
            )
            nc.sync.dma_start(
                kt_slots[s][0:HD, :].rearrange("d (h i) -> d h i", h=H),
                kt_d[w].rearrange("h d i -> d h i"),
            )
            v_t = vpool.tile([NTOK, H * 65], F16, tag="v")
            nc.sync.dma_start(v_t[:], v_d[w])

            st = stps.tile([NTOK, H * NTOK], F32, tag="st")
            for h in range(H):
                nc.tensor.matmul(
                    out=st[:, NTOK * h : NTOK * (h + 1)],
                    lhsT=kt_slots[s][0 : HD + NTOK, NTOK * h : NTOK * (h + 1)],
                    rhs=qt_slots[s][0 : HD + NTOK, NTOK * h : NTOK * (h + 1)],
                    start=True,
                    stop=True,
                )
            ex = epool.tile([NTOK, H * NTOK], F16, tag="ex")
            nc.scalar.activation(ex[:], st[:], mybir.ActivationFunctionType.Exp, scale=0.125)

            pv0 = pvps.tile([NTOK, 4 * 65], F32, tag="pv0")
            pv1 = pvps.tile([NTOK, 4 * 65], F32, tag="pv1")
            for h in range(H):
                dst = pv0 if h < 4 else pv1
                m = h % 4
                nc.tensor.matmul(
                    out=dst[:, 65 * m : 65 * (m + 1)],
                    lhsT=ex[:, NTOK * h : NTOK * (h + 1)],
                    rhs=v_t[:, 65 * h : 65 * (h + 1)],
                    start=True,
                    stop=True,
                )
            rec = rpool.tile([NTOK, H], F32, tag="rec")
            nc.vector.reciprocal(
                rec[:, 0:4].rearrange("p (h o) -> p h o", o=1),
                pv0[:].rearrange("p (h c) -> p h c", c=65)[:, :, 64:65],
            )
            nc.vector.reciprocal(
                rec[:, 4:8].rearrange("p (h o) -> p h o", o=1),
                pv1[:].rearrange("p (h c) -> p h c", c=65)[:, :, 64:65],
            )
            o_t = opool.tile([NTOK, C], F32, tag="o")
            for half, pv in ((0, pv0), (1, pv1)):
                nc.vector.tensor_tensor(
                    out=o_t[:, 256 * half : 256 * (half + 1)].rearrange(
                        "p (h c) -> p h c", c=HD
                    ),
                    in0=pv[:].rearrange("p (h c) -> p h c", c=65)[:, :, 0:HD],
                    in1=rec[:, 4 * half : 4 * half + 4]
                    .rearrange("p (h o) -> p h o", o=1)
                    .to_broadcast([NTOK, 4, HD]),
                    op=mybir.AluOpType.mult,
                )
            nc.sync.dma_start(out_d[w], o_t[:])

    nc.finalize()
    _CACHED_NC = nc
    return nc


def kernel(q, k, v, pos_proj_w, pos_proj_b, ln1_g, ln1_b, w1, b1,
           ln2_g, ln2_b, w2, b2, ln3_g, ln3_b, w3, b3):
    q = np.ascontiguousarray(np.asarray(q, dtype=np.float32))
    k = np.ascontiguousarray(np.asarray(k, dtype=np.float32))
    v = np.ascontiguousarray(np.asarray(v, dtype=np.float32))

    ident8 = np.tile(np.eye(NTOK, dtype=np.float16), (1, H))
    relidx = np.ascontiguousarray(_rel_idx().T.astype(np.int32))  # [j, i]
    biases_t = _biases_t()

    shared = {
        "ident8": ident8, "relidx": relidx, "biases_t": biases_t,
        "pos_proj_w": np.asarray(pos_proj_w, np.float32),
        "pos_proj_b": np.asarray(pos_proj_b, np.float32),
        "ln1_g": np.asarray(ln1_g, np.float32), "ln1_b": np.asarray(ln1_b, np.float32),
        "w1": np.asarray(w1, np.float32), "b1": np.asarray(b1, np.float32),
        "ln2_g": np.asarray(ln2_g, np.float32), "ln2_b": np.asarray(ln2_b, np.float32),
        "w2": np.asarray(w2, np.float32), "b2": np.asarray(b2, np.float32),
        "ln3_g": np.asarray(ln3_g, np.float32), "ln3_b": np.asarray(ln3_b, np.float32),
        "w3": np.asarray(w3, np.float32), "b3": np.asarray(b3, np.float32),
    }

    ones_col = np.ones((W, NTOK, H, 1), dtype=np.float32)
    in_maps = []
    for c in range(NCORES):
        sl = slice(c * W, (c + 1) * W)
        qt = np.ascontiguousarray(q[sl].reshape(W, NTOK, H, HD).transpose(0, 2, 3, 1).astype(np.float16))
        kt = np.ascontiguousarray(k[sl].reshape(W, NTOK, H, HD).transpose(0, 2, 3, 1).astype(np.float16))
        vaug = np.concatenate(
            [v[sl].reshape(W, NTOK, H, HD), ones_col], axis=3
        ).reshape(W, NTOK, H * 65).astype(np.float16)
        m = dict(shared)
        m.update({"qt": qt, "kt": kt, "vaug": np.ascontiguousarray(vaug)})
        in_maps.append(m)

    nc = _build_nc()
    res = run_bass_kernel_spmd(nc, in_maps, core_ids=list(range(NCORES)))
    global LAST_RESULTS
    LAST_RESULTS = res
    out = np.concatenate([r["out"] for r in res.results], axis=0)
    return out.reshape(B, NTOK, C)



# revision 15
# speedup vs baseline: 2.8540x; 2.8540x over previous
"""Windowed attention (swin-style, 49-token windows, 8 heads) with DynamicPosBias.

Data-parallel over B=2048 windows -> 256 windows/core on 8 cores.

Device does only the attention core; everything cheap runs on host:
- host computes the DynamicPosBias MLP (169x32, microseconds) and ships
  E = exp(rpb) as a [128, 196] f16 table (multiplicative softmax bias),
  zero outside the valid key rows.
- host pre-transposes q, k, v; host normalizes the unnormalized device
  output using the denominator column the device emits per head.

Key-row convention within a head pair (partition axis): even head keys at
rows 0:49, odd head keys at rows 64:113 (compute-engine APs must start at
32-aligned partitions; matmul stationary operands need one contiguous
free dim).

Per window on device:
- 8 QK matmuls (stationary K_h [64, 49], moving Q_h [64, 49]) into
  S^T PSUM [128, 392] (two windows share one PSUM tile; rows 49:64 and
  113:128 stay zero).
- one exp (scale 1/8, ACT) + one Pool multiply by E -> exm [128, 392] f16.
- 4 PV matmuls: stationary exm block [128, 49], moving V-pair [128, 2, 65]
  (65th column of each head's V is ones) -> out [49, 130] token-major with
  the softmax denominator in column 64 of each head block.
- f32->f16 copies of the PV PSUM split across ACT / DVE.
DMA in 16-window groups (6 dma_starts per group) issued from the sync and
scalar sequencers (~700 ns fixed issue cost each).
"""

import numpy as np
from contextlib import ExitStack

import concourse.bass as bass
import concourse.mybir as mybir
import concourse.tile as tile
from concourse import bacc
from concourse.bass_utils import run_bass_kernel_spmd

G = 7
NTOK = 49          # tokens per window
H = 8              # heads
HD = 64            # head dim
C = 512
B = 2048
NCORES = 8
W = B // NCORES    # windows per core = 256
GRP = 16           # windows per DMA group
NG = W // GRP      # 16 groups
NPAIR = GRP // 2   # window pairs per group
NEX = 3            # exm slots
F32 = mybir.dt.float32
F16 = mybir.dt.float16

_CACHED_NC = None
LAST_RESULTS = None


def _rel_idx():
    coords = np.stack(np.meshgrid(np.arange(G), np.arange(G), indexing="ij")).reshape(2, -1)
    rel = (coords[:, :, None] - coords[:, None, :]).transpose(1, 2, 0).copy()
    rel[:, :, 0] += G - 1
    rel[:, :, 1] += G - 1
    rel[:, :, 0] *= 2 * G - 1
    return rel.sum(-1)  # [t, j] in [0, 169)


def _ln(x, g, b, eps=1e-5):
    mu = x.mean(-1, keepdims=True)
    var = ((x - mu) ** 2).mean(-1, keepdims=True)
    return (x - mu) / np.sqrt(var + eps) * g + b


def _host_pos_mlp(pos_proj_w, pos_proj_b, ln1_g, ln1_b, w1, b1,
                  ln2_g, ln2_b, w2, b2, ln3_g, ln3_b, w3, b3):
    pb = np.arange(1 - G, G, dtype=np.float64)
    biases = np.stack(np.meshgrid(pb, pb, indexing="ij")).reshape(2, -1).T  # [169, 2]
    pos = biases @ pos_proj_w + pos_proj_b
    pos = np.maximum(_ln(pos, ln1_g, ln1_b), 0.0) @ w1 + b1
    pos = np.maximum(_ln(pos, ln2_g, ln2_b), 0.0) @ w2 + b2
    pos = np.maximum(_ln(pos, ln3_g, ln3_b), 0.0) @ w3 + b3  # [169, 8]
    rpb = pos[_rel_idx()]            # [49, 49, 8] = (t, j, h)
    rpbr = rpb.transpose(2, 0, 1).reshape(4, 2, NTOK, NTOK)  # (i, p, t, j)
    Eh = np.exp(rpbr).transpose(1, 3, 0, 2)  # (p, j, i, t)
    E = np.zeros((128, 4 * NTOK), np.float32)
    E[0:NTOK] = Eh[0].reshape(NTOK, 4 * NTOK)
    E[64:64 + NTOK] = Eh[1].reshape(NTOK, 4 * NTOK)
    return np.ascontiguousarray(E)


def _build_nc():
    global _CACHED_NC
    if _CACHED_NC is not None:
        return _CACHED_NC
    nc = bacc.Bacc(None, target_bir_lowering=False)

    qp_d = nc.dram_tensor("qp", [NG, GRP, 128, 196], F16, kind="ExternalInput")
    kpe_d = nc.dram_tensor("kpe", [NG, GRP, 64, 196], F16, kind="ExternalInput")
    kpo_d = nc.dram_tensor("kpo", [NG, GRP, 64, 196], F16, kind="ExternalInput")
    vpe_d = nc.dram_tensor("vpe", [NG, GRP, NTOK, 260], F16, kind="ExternalInput")
    vpo_d = nc.dram_tensor("vpo", [NG, GRP, NTOK, 260], F16, kind="ExternalInput")
    e_d = nc.dram_tensor("etab", [128, 196], F16, kind="ExternalInput")
    ot_d = nc.dram_tensor("ot", [NG, NTOK, GRP * 520], F16, kind="ExternalOutput")

    with tile.TileContext(nc) as tc, ExitStack() as ctx:
        const = ctx.enter_context(tc.tile_pool(name="const", bufs=1))
        expool = ctx.enter_context(tc.tile_pool(name="expool", bufs=NEX))
        stp = ctx.enter_context(tc.tile_pool(name="stp", bufs=1, space="PSUM"))
        pvp = ctx.enter_context(tc.tile_pool(name="pvp", bufs=2, space="PSUM"))

        e_sb = const.tile([128, 196], F16, tag="etab")
        nc.sync.dma_start(e_sb[:], e_d[:])

        # two persistent S^T PSUM tiles; rows 49:64, 113:128 are never
        # written by the QK matmuls and must read as zero for exp
        st_ab = []
        for s in range(2):
            st = stp.tile([128, 392], F32, tag=f"st{s}", name=f"st{s}")
            nc.vector.memset(st[:], 0.0)
            st_ab.append(st)

        qt_s, kt_s, vt_s, ot_s = [], [], [], []
        for s in range(2):
            qt = const.tile([128, GRP * 196], F16, tag=f"qt{s}", name=f"qt{s}")
            kt = const.tile([128, GRP * 392], F16, tag=f"kt{s}", name=f"kt{s}")
            vt = const.tile([128, GRP * 520], F16, tag=f"vt{s}", name=f"vt{s}")
            ot = const.tile([NTOK, GRP * 520], F16, tag=f"ot{s}", name=f"ot{s}")
            # V-pair moving operand: off-parity quadrants must be zero
            nc.gpsimd.memset(vt[:], 0.0)
            qt_s.append(qt); kt_s.append(kt); vt_s.append(vt); ot_s.append(ot)

        exm_s = [const.tile([128, 392], F16, tag=f"exm{s}", name=f"exm{s}")
                 for s in range(NEX)]

        for g in range(NG):
            s = g % 2
            qt, kt, vt, ot = qt_s[s], kt_s[s], vt_s[s], ot_s[s]
            # input DMAs for this group
            nc.sync.dma_start(
                qt[:].rearrange("p (w c) -> p w c", w=GRP),
                qp_d[g].rearrange("w p c -> p w c"))
            ktv = kt[:].rearrange("p (w h c) -> p w h c", w=GRP, h=2)
            nc.sync.dma_start(ktv[0:64, :, 0, :], kpe_d[g].rearrange("w p c -> p w c"))
            nc.sync.dma_start(ktv[64:128, :, 1, :], kpo_d[g].rearrange("w p c -> p w c"))
            vtv = vt[:].rearrange("p (w h c) -> p w h c", w=GRP, h=2)
            nc.scalar.dma_start(vtv[0:NTOK, :, 0, :], vpe_d[g].rearrange("w p c -> p w c"))
            nc.scalar.dma_start(vtv[64:64 + NTOK, :, 1, :],
                                vpo_d[g].rearrange("w p c -> p w c"))

            ktq = kt[:].rearrange("p (w h i t) -> p w h i t", w=GRP, h=2, i=4)
            vtq = vt[:].rearrange("p (w h i c) -> p w i h c", w=GRP, h=2, i=4)

            for wp in range(NPAIR):
                st = st_ab[wp % 2]
                for w01 in (0, 1):
                    w = 2 * wp + w01
                    for i in range(4):
                        nc.tensor.matmul(
                            out=st[0:NTOK, w01 * 196 + 49 * i: w01 * 196 + 49 * (i + 1)],
                            lhsT=ktq[0:64, w, 0, i, :],
                            rhs=qt[0:64, w * 196 + 49 * i: w * 196 + 49 * (i + 1)],
                            start=True, stop=True)
                        nc.tensor.matmul(
                            out=st[64:64 + NTOK, w01 * 196 + 49 * i: w01 * 196 + 49 * (i + 1)],
                            lhsT=ktq[64:128, w, 1, i, :],
                            rhs=qt[64:128, w * 196 + 49 * i: w * 196 + 49 * (i + 1)],
                            start=True, stop=True)
                ex = expool.tile([128, 392], F16, tag="ex")
                nc.scalar.activation(ex[:], st[:], mybir.ActivationFunctionType.Exp,
                                     scale=0.125)
                exm = exm_s[(g * NPAIR + wp) % NEX]
                nc.gpsimd.tensor_tensor(out=exm[:, 0:196], in0=ex[:, 0:196],
                                        in1=e_sb[:], op=mybir.AluOpType.mult)
                nc.gpsimd.tensor_tensor(out=exm[:, 196:392], in0=ex[:, 196:392],
                                        in1=e_sb[:], op=mybir.AluOpType.mult)

                for w01 in (0, 1):
                    w = 2 * wp + w01
                    pva = pvp.tile([NTOK, 260], F32, tag="pva")
                    pvb = pvp.tile([NTOK, 260], F32, tag="pvb")
                    for i in range(4):
                        dst = pva if i < 2 else pvb
                        nc.tensor.matmul(
                            out=dst[:, 130 * (i % 2): 130 * (i % 2 + 1)],
                            lhsT=exm[:, w01 * 196 + 49 * i: w01 * 196 + 49 * (i + 1)],
                            rhs=vtq[:, w, i, :, :],
                            start=True, stop=True)
                    # f32 -> f16 copies, split across ACT / DVE
                    nc.scalar.copy(ot[:, w * 520: w * 520 + 260], pva[:])
                    nc.vector.tensor_copy(ot[:, w * 520 + 260: (w + 1) * 520], pvb[:])

            nc.scalar.dma_start(ot_d[g], ot[:])

    nc.finalize()
    _CACHED_NC = nc
    return nc


def kernel(q, k, v, pos_proj_w, pos_proj_b, ln1_g, ln1_b, w1, b1,
           ln2_g, ln2_b, w2, b2, ln3_g, ln3_b, w3, b3):
    q = np.asarray(q, dtype=np.float32)
    k = np.asarray(k, dtype=np.float32)
    v = np.asarray(v, dtype=np.float32)

    E = _host_pos_mlp(
        np.asarray(pos_proj_w, np.float64), np.asarray(pos_proj_b, np.float64),
        np.asarray(ln1_g, np.float64), np.asarray(ln1_b, np.float64),
        np.asarray(w1, np.float64), np.asarray(b1, np.float64),
        np.asarray(ln2_g, np.float64), np.asarray(ln2_b, np.float64),
        np.asarray(w2, np.float64), np.asarray(b2, np.float64),
        np.asarray(ln3_g, np.float64), np.asarray(ln3_b, np.float64),
        np.asarray(w3, np.float64), np.asarray(b3, np.float64)).astype(np.float16)

    in_maps = []
    for c in range(NCORES):
        sl = slice(c * W, (c + 1) * W)
        qh = q[sl].reshape(W, NTOK, 4, 2, HD)   # (w, t, i, p, d)
        qp = np.ascontiguousarray(
            qh.transpose(0, 3, 4, 2, 1).reshape(NG, GRP, 128, 196).astype(np.float16))
        kh = k[sl].reshape(W, NTOK, 4, 2, HD)
        kpe = np.ascontiguousarray(
            kh[:, :, :, 0, :].transpose(0, 3, 2, 1).reshape(NG, GRP, 64, 196).astype(np.float16))
        kpo = np.ascontiguousarray(
            kh[:, :, :, 1, :].transpose(0, 3, 2, 1).reshape(NG, GRP, 64, 196).astype(np.float16))
        vh = v[sl].reshape(W, NTOK, 4, 2, HD)   # (w, j, i, p, d)
        ones = np.ones((W, NTOK, 4, 1), np.float32)
        vpe = np.ascontiguousarray(np.concatenate(
            [vh[:, :, :, 0, :], ones], axis=3).reshape(NG, GRP, NTOK, 260).astype(np.float16))
        vpo = np.ascontiguousarray(np.concatenate(
            [vh[:, :, :, 1, :], ones], axis=3).reshape(NG, GRP, NTOK, 260).astype(np.float16))
        in_maps.append({"qp": qp, "kpe": kpe, "kpo": kpo, "vpe": vpe, "vpo": vpo,
                        "etab": E})

    nc = _build_nc()
    res = run_bass_kernel_spmd(nc, in_maps, core_ids=list(range(NCORES)))
    global LAST_RESULTS
    LAST_RESULTS = res

    outs = []
    for r in res.results:
        ot = np.asarray(r["ot"]).reshape(NG, NTOK, GRP, 520)
        ot = ot.transpose(0, 2, 1, 3).reshape(W, NTOK, 4, 2, 65).astype(np.float32)
        num = ot[..., 0:64]                    # (w, t, i, p, d)
        den = ot[..., 64:65]
        outs.append((num / den).reshape(W, NTOK, C))
    return np.ascontiguousarray(np.concatenate(outs, axis=0))


# revision 16
# speedup vs baseline: 3.1068x; 1.0886x over previous
"""Windowed attention (swin-style, 49-token windows, 8 heads) with DynamicPosBias.

Data-parallel over B=2048 windows -> 256 windows/core on 8 cores.

Device does only the attention core; everything cheap runs on host:
- host computes the DynamicPosBias MLP (169x32, microseconds) and ships
  E = exp(rpb) as a [128, 196] f16 table (multiplicative softmax bias),
  zero outside the valid key rows.
- host pre-transposes q, k, v; host normalizes the unnormalized device
  output using the denominator column the device emits per head.

Key-row convention within a head pair (partition axis): even head keys at
rows 0:49, odd head keys at rows 64:113 (compute-engine APs must start at
32-aligned partitions; matmul stationary operands need one contiguous
free dim).

Per window on device:
- 8 QK matmuls (stationary K_h [64, 49], moving Q_h [64, 49]) into
  S^T PSUM [128, 392] (two windows share one PSUM tile; rows 49:64 and
  113:128 stay zero).
- one exp (scale 1/8, ACT) + one Pool multiply by E -> exm [128, 392] f16.
- 4 PV matmuls: stationary exm block [128, 49], moving V-pair [128, 2, 65]
  (65th column of each head's V is ones) -> out [49, 130] token-major with
  the softmax denominator in column 64 of each head block.
- f32->f16 copies of the PV PSUM split across ACT / DVE.
DMA in 16-window groups (6 dma_starts per group) issued from the sync and
scalar sequencers (~700 ns fixed issue cost each).
"""

import numpy as np
from contextlib import ExitStack

import concourse.bass as bass
import concourse.mybir as mybir
import concourse.tile as tile
from concourse import bacc
from concourse.bass_utils import run_bass_kernel_spmd

G = 7
NTOK = 49          # tokens per window
H = 8              # heads
HD = 64            # head dim
C = 512
B = 2048
NCORES = 8
W = B // NCORES    # windows per core = 256
GRP = 16           # windows per DMA group
NG = W // GRP      # 16 groups
NPAIR = GRP // 2   # window pairs per group
NEX = 3            # exm slots
F32 = mybir.dt.float32
F16 = mybir.dt.float16

_CACHED_NC = None
LAST_RESULTS = None


def _rel_idx():
    coords = np.stack(np.meshgrid(np.arange(G), np.arange(G), indexing="ij")).reshape(2, -1)
    rel = (coords[:, :, None] - coords[:, None, :]).transpose(1, 2, 0).copy()
    rel[:, :, 0] += G - 1
    rel[:, :, 1] += G - 1
    rel[:, :, 0] *= 2 * G - 1
    return rel.sum(-1)  # [t, j] in [0, 169)


def _ln(x, g, b, eps=1e-5):
    mu = x.mean(-1, keepdims=True)
    var = ((x - mu) ** 2).mean(-1, keepdims=True)
    return (x - mu) / np.sqrt(var + eps) * g + b


def _host_pos_mlp(pos_proj_w, pos_proj_b, ln1_g, ln1_b, w1, b1,
                  ln2_g, ln2_b, w2, b2, ln3_g, ln3_b, w3, b3):
    pb = np.arange(1 - G, G, dtype=np.float64)
    biases = np.stack(np.meshgrid(pb, pb, indexing="ij")).reshape(2, -1).T  # [169, 2]
    pos = biases @ pos_proj_w + pos_proj_b
    pos = np.maximum(_ln(pos, ln1_g, ln1_b), 0.0) @ w1 + b1
    pos = np.maximum(_ln(pos, ln2_g, ln2_b), 0.0) @ w2 + b2
    pos = np.maximum(_ln(pos, ln3_g, ln3_b), 0.0) @ w3 + b3  # [169, 8]
    rpb = pos[_rel_idx()]            # [49, 49, 8] = (t, j, h)
    rpbr = rpb.transpose(2, 0, 1).reshape(4, 2, NTOK, NTOK)  # (i, p, t, j)
    Eh = np.exp(rpbr).transpose(1, 3, 0, 2)  # (p, j, i, t)
    E = np.zeros((128, 4 * NTOK), np.float32)
    E[0:NTOK] = Eh[0].reshape(NTOK, 4 * NTOK)
    E[64:64 + NTOK] = Eh[1].reshape(NTOK, 4 * NTOK)
    return np.ascontiguousarray(E)


def _build_nc():
    global _CACHED_NC
    if _CACHED_NC is not None:
        return _CACHED_NC
    nc = bacc.Bacc(None, target_bir_lowering=False)

    qp_d = nc.dram_tensor("qp", [NG, 128, GRP * 196], F16, kind="ExternalInput")
    kpe_d = nc.dram_tensor("kpe", [NG, 64, GRP * 196], F16, kind="ExternalInput")
    kpo_d = nc.dram_tensor("kpo", [NG, 64, GRP * 196], F16, kind="ExternalInput")
    vpe_d = nc.dram_tensor("vpe", [NG, NTOK, GRP * 260], F16, kind="ExternalInput")
    vpo_d = nc.dram_tensor("vpo", [NG, NTOK, GRP * 260], F16, kind="ExternalInput")
    e_d = nc.dram_tensor("etab", [128, 196], F16, kind="ExternalInput")
    ot_d = nc.dram_tensor("ot", [NG, NTOK, GRP * 520], F16, kind="ExternalOutput")

    with tile.TileContext(nc) as tc, ExitStack() as ctx:
        const = ctx.enter_context(tc.tile_pool(name="const", bufs=1))
        expool = ctx.enter_context(tc.tile_pool(name="expool", bufs=NEX))
        stp = ctx.enter_context(tc.tile_pool(name="stp", bufs=1, space="PSUM"))
        pvp = ctx.enter_context(tc.tile_pool(name="pvp", bufs=2, space="PSUM"))

        e_sb = const.tile([128, 196], F16, tag="etab")
        nc.sync.dma_start(e_sb[:], e_d[:])

        # two persistent S^T PSUM tiles; rows 49:64, 113:128 are never
        # written by the QK matmuls and must read as zero for exp
        st_ab = []
        for s in range(2):
            st = stp.tile([128, 392], F32, tag=f"st{s}", name=f"st{s}")
            nc.vector.memset(st[:], 0.0)
            st_ab.append(st)

        qt_s, kt_s, vt_s, ot_s = [], [], [], []
        for s in range(2):
            qt = const.tile([128, GRP * 196], F16, tag=f"qt{s}", name=f"qt{s}")
            kt = const.tile([128, 2 * GRP * 196], F16, tag=f"kt{s}", name=f"kt{s}")
            vt = const.tile([128, 2 * GRP * 260], F16, tag=f"vt{s}", name=f"vt{s}")
            ot = const.tile([NTOK, GRP * 520], F16, tag=f"ot{s}", name=f"ot{s}")
            # V-pair moving operand: off-parity quadrants must be zero
            nc.gpsimd.memset(vt[:], 0.0)
            qt_s.append(qt); kt_s.append(kt); vt_s.append(vt); ot_s.append(ot)

        exm_s = [const.tile([128, 392], F16, tag=f"exm{s}", name=f"exm{s}")
                 for s in range(NEX)]

        for g in range(NG):
            s = g % 2
            qt, kt, vt, ot = qt_s[s], kt_s[s], vt_s[s], ot_s[s]
            # input DMAs: split into chunks so descriptors fan out across
            # the 16 DMA queues; all runs are multi-KB contiguous
            QC = GRP * 196 // 4
            for c4 in range(4):
                nc.sync.dma_start(qt[:, c4 * QC: (c4 + 1) * QC],
                                  qp_d[g][:, c4 * QC: (c4 + 1) * QC])
            KC = GRP * 196 // 2
            for c2 in range(2):
                nc.sync.dma_start(kt[0:64, c2 * KC: (c2 + 1) * KC],
                                  kpe_d[g][:, c2 * KC: (c2 + 1) * KC])
                nc.sync.dma_start(kt[64:128, GRP * 196 + c2 * KC: GRP * 196 + (c2 + 1) * KC],
                                  kpo_d[g][:, c2 * KC: (c2 + 1) * KC])
            VC = GRP * 260 // 2
            for c2 in range(2):
                nc.scalar.dma_start(vt[0:NTOK, c2 * VC: (c2 + 1) * VC],
                                    vpe_d[g][:, c2 * VC: (c2 + 1) * VC])
                nc.scalar.dma_start(vt[64:64 + NTOK, GRP * 260 + c2 * VC: GRP * 260 + (c2 + 1) * VC],
                                    vpo_d[g][:, c2 * VC: (c2 + 1) * VC])

            ktq = kt[:].rearrange("p (h w i t) -> p h w i t", h=2, w=GRP, i=4)
            vtq = vt[:].rearrange("p (h w i c) -> p w i h c", h=2, w=GRP, i=4)

            for wp in range(NPAIR):
                st = st_ab[wp % 2]
                for w01 in (0, 1):
                    w = 2 * wp + w01
                    for i in range(4):
                        nc.tensor.matmul(
                            out=st[0:NTOK, w01 * 196 + 49 * i: w01 * 196 + 49 * (i + 1)],
                            lhsT=ktq[0:64, 0, w, i, :],
                            rhs=qt[0:64, w * 196 + 49 * i: w * 196 + 49 * (i + 1)],
                            start=True, stop=True)
                        nc.tensor.matmul(
                            out=st[64:64 + NTOK, w01 * 196 + 49 * i: w01 * 196 + 49 * (i + 1)],
                            lhsT=ktq[64:128, 1, w, i, :],
                            rhs=qt[64:128, w * 196 + 49 * i: w * 196 + 49 * (i + 1)],
                            start=True, stop=True)
                ex = expool.tile([128, 392], F16, tag="ex")
                nc.scalar.activation(ex[:], st[:], mybir.ActivationFunctionType.Exp,
                                     scale=0.125)
                exm = exm_s[(g * NPAIR + wp) % NEX]
                nc.gpsimd.tensor_tensor(out=exm[:, 0:196], in0=ex[:, 0:196],
                                        in1=e_sb[:], op=mybir.AluOpType.mult)
                nc.gpsimd.tensor_tensor(out=exm[:, 196:392], in0=ex[:, 196:392],
                                        in1=e_sb[:], op=mybir.AluOpType.mult)

                for w01 in (0, 1):
                    w = 2 * wp + w01
                    pva = pvp.tile([NTOK, 260], F32, tag="pva")
                    pvb = pvp.tile([NTOK, 260], F32, tag="pvb")
                    for i in range(4):
                        dst = pva if i < 2 else pvb
                        nc.tensor.matmul(
                            out=dst[:, 130 * (i % 2): 130 * (i % 2 + 1)],
                            lhsT=exm[:, w01 * 196 + 49 * i: w01 * 196 + 49 * (i + 1)],
                            rhs=vtq[:, w, i, :, :],
                            start=True, stop=True)
                    # f32 -> f16 copies, split across ACT / DVE
                    nc.scalar.copy(ot[:, w * 520: w * 520 + 260], pva[:])
                    nc.vector.tensor_copy(ot[:, w * 520 + 260: (w + 1) * 520], pvb[:])

            OC = GRP * 520 // 4
            for c4 in range(4):
                nc.gpsimd.dma_start(ot_d[g][:, c4 * OC: (c4 + 1) * OC],
                                    ot[:, c4 * OC: (c4 + 1) * OC])

    nc.finalize()
    _CACHED_NC = nc
    return nc


def kernel(q, k, v, pos_proj_w, pos_proj_b, ln1_g, ln1_b, w1, b1,
           ln2_g, ln2_b, w2, b2, ln3_g, ln3_b, w3, b3):
    q = np.asarray(q, dtype=np.float32)
    k = np.asarray(k, dtype=np.float32)
    v = np.asarray(v, dtype=np.float32)

    E = _host_pos_mlp(
        np.asarray(pos_proj_w, np.float64), np.asarray(pos_proj_b, np.float64),
        np.asarray(ln1_g, np.float64), np.asarray(ln1_b, np.float64),
        np.asarray(w1, np.float64), np.asarray(b1, np.float64),
        np.asarray(ln2_g, np.float64), np.asarray(ln2_b, np.float64),
        np.asarray(w2, np.float64), np.asarray(b2, np.float64),
        np.asarray(ln3_g, np.float64), np.asarray(ln3_b, np.float64),
        np.asarray(w3, np.float64), np.asarray(b3, np.float64)).astype(np.float16)

    in_maps = []
    for c in range(NCORES):
        sl = slice(c * W, (c + 1) * W)
        qh = q[sl].reshape(W, NTOK, 4, 2, HD)   # (w, t, i, p, d)
        qp = np.ascontiguousarray(
            qh.transpose(0, 3, 4, 2, 1).reshape(NG, GRP, 128, 196)
            .transpose(0, 2, 1, 3).reshape(NG, 128, GRP * 196).astype(np.float16))
        kh = k[sl].reshape(W, NTOK, 4, 2, HD)
        kpe = np.ascontiguousarray(
            kh[:, :, :, 0, :].transpose(0, 3, 2, 1).reshape(NG, GRP, 64, 196)
            .transpose(0, 2, 1, 3).reshape(NG, 64, GRP * 196).astype(np.float16))
        kpo = np.ascontiguousarray(
            kh[:, :, :, 1, :].transpose(0, 3, 2, 1).reshape(NG, GRP, 64, 196)
            .transpose(0, 2, 1, 3).reshape(NG, 64, GRP * 196).astype(np.float16))
        vh = v[sl].reshape(W, NTOK, 4, 2, HD)   # (w, j, i, p, d)
        ones = np.ones((W, NTOK, 4, 1), np.float32)
        vpe = np.ascontiguousarray(np.concatenate(
            [vh[:, :, :, 0, :], ones], axis=3).reshape(NG, GRP, NTOK, 260)
            .transpose(0, 2, 1, 3).reshape(NG, NTOK, GRP * 260).astype(np.float16))
        vpo = np.ascontiguousarray(np.concatenate(
            [vh[:, :, :, 1, :], ones], axis=3).reshape(NG, GRP, NTOK, 260)
            .transpose(0, 2, 1, 3).reshape(NG, NTOK, GRP * 260).astype(np.float16))
        in_maps.append({"qp": qp, "kpe": kpe, "kpo": kpo, "vpe": vpe, "vpo": vpo,
                        "etab": E})

    nc = _build_nc()
    res = run_bass_kernel_spmd(nc, in_maps, core_ids=list(range(NCORES)))
    global LAST_RESULTS
    LAST_RESULTS = res

    outs = []
    for r in res.results:
        ot = np.asarray(r["ot"]).reshape(NG, NTOK, GRP, 520)
        ot = ot.transpose(0, 2, 1, 3).reshape(W, NTOK, 4, 2, 65).astype(np.float32)
        num = ot[..., 0:64]                    # (w, t, i, p, d)
        den = ot[..., 64:65]
        outs.append((num / den).reshape(W, NTOK, C))
    return np.ascontiguousarray(np.concatenate(outs, axis=0))


# revision 18
# speedup vs baseline: 3.2608x; 1.0496x over previous
"""Windowed attention (swin-style, 49-token windows, 8 heads) with DynamicPosBias.

Data-parallel over B=2048 windows -> 256 windows/core on 8 cores.

Device does only the attention core; everything cheap runs on host:
- host computes the DynamicPosBias MLP (169x32, microseconds) and ships
  E = exp(rpb) as a [128, 196] f16 table (multiplicative softmax bias),
  zero outside the valid key rows.
- host pre-transposes q, k, v; host normalizes the unnormalized device
  output using the denominator column the device emits per head.

Key-row convention within a head pair (partition axis): even head keys at
rows 0:49, odd head keys at rows 64:113 (compute-engine APs must start at
32-aligned partitions; matmul stationary operands need one contiguous
free dim).

Per window on device:
- 8 QK matmuls (stationary K_h [64, 49], moving Q_h [64, 49]) into
  S^T PSUM [128, 392] (two windows share one PSUM tile; rows 49:64 and
  113:128 stay zero).
- one exp (scale 1/8, ACT) + one Pool multiply by E -> exm [128, 392] f16.
- 4 PV matmuls: stationary exm block [128, 49], moving V-pair [128, 2, 65]
  (65th column of each head's V is ones) -> out [49, 130] token-major with
  the softmax denominator in column 64 of each head block.
- f32->f16 copies of the PV PSUM split across ACT / DVE.
DMA in 16-window groups (6 dma_starts per group) issued from the sync and
scalar sequencers (~700 ns fixed issue cost each).
"""

import numpy as np
from contextlib import ExitStack

import concourse.bass as bass
import concourse.mybir as mybir
import concourse.tile as tile
from concourse import bacc
from concourse.bass_utils import run_bass_kernel_spmd

G = 7
NTOK = 49          # tokens per window
H = 8              # heads
HD = 64            # head dim
C = 512
B = 2048
NCORES = 8
W = B // NCORES    # windows per core = 256
GRP = 16           # windows per DMA group
NG = W // GRP      # 16 groups
NPAIR = GRP // 2   # window pairs per group
NEX = 3            # exm slots
F32 = mybir.dt.float32
F16 = mybir.dt.float16

_CACHED_NC = None
LAST_RESULTS = None


def _rel_idx():
    coords = np.stack(np.meshgrid(np.arange(G), np.arange(G), indexing="ij")).reshape(2, -1)
    rel = (coords[:, :, None] - coords[:, None, :]).transpose(1, 2, 0).copy()
    rel[:, :, 0] += G - 1
    rel[:, :, 1] += G - 1
    rel[:, :, 0] *= 2 * G - 1
    return rel.sum(-1)  # [t, j] in [0, 169)


def _ln(x, g, b, eps=1e-5):
    mu = x.mean(-1, keepdims=True)
    var = ((x - mu) ** 2).mean(-1, keepdims=True)
    return (x - mu) / np.sqrt(var + eps) * g + b


def _host_pos_mlp(pos_proj_w, pos_proj_b, ln1_g, ln1_b, w1, b1,
                  ln2_g, ln2_b, w2, b2, ln3_g, ln3_b, w3, b3):
    pb = np.arange(1 - G, G, dtype=np.float64)
    biases = np.stack(np.meshgrid(pb, pb, indexing="ij")).reshape(2, -1).T  # [169, 2]
    pos = biases @ pos_proj_w + pos_proj_b
    pos = np.maximum(_ln(pos, ln1_g, ln1_b), 0.0) @ w1 + b1
    pos = np.maximum(_ln(pos, ln2_g, ln2_b), 0.0) @ w2 + b2
    pos = np.maximum(_ln(pos, ln3_g, ln3_b), 0.0) @ w3 + b3  # [169, 8]
    rpb = pos[_rel_idx()]            # [49, 49, 8] = (t, j, h)
    rpbr = rpb.transpose(2, 0, 1).reshape(4, 2, NTOK, NTOK)  # (i, p, t, j)
    Eh = np.exp(rpbr).transpose(1, 3, 0, 2)  # (p, j, i, t)
    E = np.zeros((128, 4 * NTOK), np.float32)
    E[0:NTOK] = Eh[0].reshape(NTOK, 4 * NTOK)
    E[64:64 + NTOK] = Eh[1].reshape(NTOK, 4 * NTOK)
    return np.ascontiguousarray(E)


def _build_nc():
    global _CACHED_NC
    if _CACHED_NC is not None:
        return _CACHED_NC
    nc = bacc.Bacc(None, target_bir_lowering=False)

    qp_d = nc.dram_tensor("qp", [NG, 128, GRP * 196], F16, kind="ExternalInput")
    kpe_d = nc.dram_tensor("kpe", [NG, 64, GRP * 196], F16, kind="ExternalInput")
    kpo_d = nc.dram_tensor("kpo", [NG, 64, GRP * 196], F16, kind="ExternalInput")
    vpe_d = nc.dram_tensor("vpe", [NG, NTOK, GRP * 260], F16, kind="ExternalInput")
    vpo_d = nc.dram_tensor("vpo", [NG, NTOK, GRP * 260], F16, kind="ExternalInput")
    e_d = nc.dram_tensor("etab", [128, 196], F16, kind="ExternalInput")
    ot_d = nc.dram_tensor("ot", [NG, NTOK, GRP * 520], F16, kind="ExternalOutput")

    with tile.TileContext(nc) as tc, ExitStack() as ctx:
        const = ctx.enter_context(tc.tile_pool(name="const", bufs=1))
        expool = ctx.enter_context(tc.tile_pool(name="expool", bufs=NEX))
        stp = ctx.enter_context(tc.tile_pool(name="stp", bufs=1, space="PSUM"))
        pvp = ctx.enter_context(tc.tile_pool(name="pvp", bufs=3, space="PSUM"))

        e_sb = const.tile([128, 196], F16, tag="etab")
        nc.sync.dma_start(e_sb[:], e_d[:])

        # two persistent S^T PSUM tiles; rows 49:64, 113:128 are never
        # written by the QK matmuls and must read as zero for exp
        st_ab = []
        for s in range(2):
            st = stp.tile([128, 392], F32, tag=f"st{s}", name=f"st{s}")
            nc.vector.memset(st[:], 0.0)
            st_ab.append(st)

        qt_s, kt_s, vt_s, ot_s = [], [], [], []
        for s in range(2):
            qt = const.tile([128, GRP * 196], F16, tag=f"qt{s}", name=f"qt{s}")
            kt = const.tile([128, 2 * GRP * 196], F16, tag=f"kt{s}", name=f"kt{s}")
            vt = const.tile([128, 2 * GRP * 260], F16, tag=f"vt{s}", name=f"vt{s}")
            ot = const.tile([NTOK, GRP * 520], F16, tag=f"ot{s}", name=f"ot{s}")
            # V-pair moving operand: off-parity quadrants must be zero
            nc.gpsimd.memset(vt[:], 0.0)
            qt_s.append(qt); kt_s.append(kt); vt_s.append(vt); ot_s.append(ot)

        exm_s = [const.tile([128, 392], F16, tag=f"exm{s}", name=f"exm{s}")
                 for s in range(NEX)]

        ktq_s = [None, None]
        vtq_s = [None, None]

        def _emit_pv(p):
            gg, wpp = divmod(p, NPAIR)
            ss = gg % 2
            ott = ot_s[ss]
            exm = exm_s[p % NEX]
            for w01 in (0, 1):
                w = 2 * wpp + w01
                pva = pvp.tile([NTOK, 260], F32, tag="pva")
                pvb = pvp.tile([NTOK, 260], F32, tag="pvb")
                for i in range(4):
                    dst = pva if i < 2 else pvb
                    nc.tensor.matmul(
                        out=dst[:, 130 * (i % 2): 130 * (i % 2 + 1)],
                        lhsT=exm[:, w01 * 196 + 49 * i: w01 * 196 + 49 * (i + 1)],
                        rhs=vtq_s[ss][:, w, i, :, :],
                        start=True, stop=True)
                # f32 -> f16 copies, split across ACT / DVE
                nc.scalar.copy(ott[:, w * 520: w * 520 + 260], pva[:])
                nc.vector.tensor_copy(ott[:, w * 520 + 260: (w + 1) * 520], pvb[:])
            if wpp == NPAIR - 1:
                OC = GRP * 520 // 4
                for c4 in range(4):
                    nc.gpsimd.dma_start(ot_d[gg][:, c4 * OC: (c4 + 1) * OC],
                                        ott[:, c4 * OC: (c4 + 1) * OC])

        for g in range(NG):
            s = g % 2
            qt, kt, vt, ot = qt_s[s], kt_s[s], vt_s[s], ot_s[s]
            # input DMAs: split into chunks so descriptors fan out across
            # the 16 DMA queues; all runs are multi-KB contiguous
            QC = GRP * 196 // 4
            for c4 in range(4):
                nc.sync.dma_start(qt[:, c4 * QC: (c4 + 1) * QC],
                                  qp_d[g][:, c4 * QC: (c4 + 1) * QC])
            KC = GRP * 196 // 2
            for c2 in range(2):
                nc.sync.dma_start(kt[0:64, c2 * KC: (c2 + 1) * KC],
                                  kpe_d[g][:, c2 * KC: (c2 + 1) * KC])
                nc.sync.dma_start(kt[64:128, GRP * 196 + c2 * KC: GRP * 196 + (c2 + 1) * KC],
                                  kpo_d[g][:, c2 * KC: (c2 + 1) * KC])
            VC = GRP * 260 // 2
            for c2 in range(2):
                nc.scalar.dma_start(vt[0:NTOK, c2 * VC: (c2 + 1) * VC],
                                    vpe_d[g][:, c2 * VC: (c2 + 1) * VC])
                nc.scalar.dma_start(vt[64:64 + NTOK, GRP * 260 + c2 * VC: GRP * 260 + (c2 + 1) * VC],
                                    vpo_d[g][:, c2 * VC: (c2 + 1) * VC])

            ktq_s[s] = kt[:].rearrange("p (h w i t) -> p h w i t", h=2, w=GRP, i=4)
            vtq_s[s] = vt[:].rearrange("p (h w i c) -> p w i h c", h=2, w=GRP, i=4)

            for wp in range(NPAIR):
                p = g * NPAIR + wp
                st = st_ab[p % 2]
                for w01 in (0, 1):
                    w = 2 * wp + w01
                    for i in range(4):
                        nc.tensor.matmul(
                            out=st[0:NTOK, w01 * 196 + 49 * i: w01 * 196 + 49 * (i + 1)],
                            lhsT=ktq_s[s][0:64, 0, w, i, :],
                            rhs=qt[0:64, w * 196 + 49 * i: w * 196 + 49 * (i + 1)],
                            start=True, stop=True)
                        nc.tensor.matmul(
                            out=st[64:64 + NTOK, w01 * 196 + 49 * i: w01 * 196 + 49 * (i + 1)],
                            lhsT=ktq_s[s][64:128, 1, w, i, :],
                            rhs=qt[64:128, w * 196 + 49 * i: w * 196 + 49 * (i + 1)],
                            start=True, stop=True)
                ex = expool.tile([128, 392], F16, tag="ex")
                nc.scalar.activation(ex[:], st[:], mybir.ActivationFunctionType.Exp,
                                     scale=0.125)
                exm = exm_s[p % NEX]
                nc.vector.tensor_tensor(
                    out=exm[:].rearrange("p (o c) -> p o c", o=2),
                    in0=ex[:].rearrange("p (o c) -> p o c", o=2),
                    in1=e_sb[:].rearrange("p (o c) -> p o c", o=1).to_broadcast([128, 2, 196]),
                    op=mybir.AluOpType.mult)
                # software pipelining: emit PV for the PREVIOUS pair so the
                # in-order PE never stalls waiting for this pair's exp/mult
                if p > 0:
                    _emit_pv(p - 1)
            # after the last pair of this group, PV for it is still pending;
            # it is emitted in the next group (or flushed after the loop)

        _emit_pv(NG * NPAIR - 1)


    nc.finalize()
    _CACHED_NC = nc
    return nc


def kernel(q, k, v, pos_proj_w, pos_proj_b, ln1_g, ln1_b, w1, b1,
           ln2_g, ln2_b, w2, b2, ln3_g, ln3_b, w3, b3):
    q = np.asarray(q, dtype=np.float32)
    k = np.asarray(k, dtype=np.float32)
    v = np.asarray(v, dtype=np.float32)

    E = _host_pos_mlp(
        np.asarray(pos_proj_w, np.float64), np.asarray(pos_proj_b, np.float64),
        np.asarray(ln1_g, np.float64), np.asarray(ln1_b, np.float64),
        np.asarray(w1, np.float64), np.asarray(b1, np.float64),
        np.asarray(ln2_g, np.float64), np.asarray(ln2_b, np.float64),
        np.asarray(w2, np.float64), np.asarray(b2, np.float64),
        np.asarray(ln3_g, np.float64), np.asarray(ln3_b, np.float64),
        np.asarray(w3, np.float64), np.asarray(b3, np.float64)).astype(np.float16)

    in_maps = []
    for c in range(NCORES):
        sl = slice(c * W, (c + 1) * W)
        qh = q[sl].reshape(W, NTOK, 4, 2, HD)   # (w, t, i, p, d)
        qp = np.ascontiguousarray(
            qh.transpose(0, 3, 4, 2, 1).reshape(NG, GRP, 128, 196)
            .transpose(0, 2, 1, 3).reshape(NG, 128, GRP * 196).astype(np.float16))
        kh = k[sl].reshape(W, NTOK, 4, 2, HD)
        kpe = np.ascontiguousarray(
            kh[:, :, :, 0, :].transpose(0, 3, 2, 1).reshape(NG, GRP, 64, 196)
            .transpose(0, 2, 1, 3).reshape(NG, 64, GRP * 196).astype(np.float16))
        kpo = np.ascontiguousarray(
            kh[:, :, :, 1, :].transpose(0, 3, 2, 1).reshape(NG, GRP, 64, 196)
            .transpose(0, 2, 1, 3).reshape(NG, 64, GRP * 196).astype(np.float16))
        vh = v[sl].reshape(W, NTOK, 4, 2, HD)   # (w, j, i, p, d)
        ones = np.ones((W, NTOK, 4, 1), np.float32)
        vpe = np.ascontiguousarray(np.concatenate(
            [vh[:, :, :, 0, :], ones], axis=3).reshape(NG, GRP, NTOK, 260)
            .transpose(0, 2, 1, 3).reshape(NG, NTOK, GRP * 260).astype(np.float16))
        vpo = np.ascontiguousarray(np.concatenate(
            [vh[:, :, :, 1, :], ones], axis=3).reshape(NG, GRP, NTOK, 260)
            .transpose(0, 2, 1, 3).reshape(NG, NTOK, GRP * 260).astype(np.float16))
        in_maps.append({"qp": qp, "kpe": kpe, "kpo": kpo, "vpe": vpe, "vpo": vpo,
                        "etab": E})

    nc = _build_nc()
    res = run_bass_kernel_spmd(nc, in_maps, core_ids=list(range(NCORES)))
    global LAST_RESULTS
    LAST_RESULTS = res

    outs = []
    for r in res.results:
        ot = np.asarray(r["ot"]).reshape(NG, NTOK, GRP, 520)
        ot = ot.transpose(0, 2, 1, 3).reshape(W, NTOK, 4, 2, 65).astype(np.float32)
        num = ot[..., 0:64]                    # (w, t, i, p, d)
        den = ot[..., 64:65]
        outs.append((num / den).reshape(W, NTOK, C))
    return np.ascontiguousarray(np.concatenate(outs, axis=0))
